# revision 7
# baseline (speedup 1.0000x reference)
"""Trainium2 Bass kernel for nn_KVEmbedding (embedding row-gather).

Problem: out[b, l, :] = table[indices[b, l], :]
  indices: (4096, 200) int64, values in [0, 1e6)
  table:   (1000000, 64) float32   (256 MB -- fits in every core's HBM)
  out:     (4096, 200, 64) float32

Sharding: table replicated on all 8 cores; lookups sharded by batch.  Each
core gathers its 102,400 rows from its local table copy via indirect DMA.
HW indirect-DMA semantics (validated empirically): ONE offset per partition
per instruction, each moving one contiguous 64-elem table row into that
partition.  So each gather instruction moves 128 rows (one per partition,
offsets = one column of the idx tile); 50 gathers fill a wide SBUF buffer
which is then written out with a single large HWDGE DMA.  Double-buffered.

  Pool (SWDGE): 800 indirect gathers, chunk c -> buffer (c//50)%2 col c%50
  SP   (HWDGE): idx load + 16 writeouts of [128, 50*64] to out DRAM
"""

import contextlib

import numpy as np

import concourse.bass as bass
import concourse.mybir as mybir
from concourse.bass_utils import run_bass_kernel_spmd

B, L, D = 4096, 200, 64
VOCAB = 1_000_000
N_CORES = 8
R = B * L // N_CORES  # 102,400 gathered rows per core
P = 128               # SBUF partitions
Q = R // P            # 800 rows per partition = 800 gather instructions
W = 50                # gather columns per writeout buffer
NBUF = 2              # writeout buffers

_compiled = None


def build(vocab=VOCAB, q=Q, w=W, nbuf=NBUF):
    assert q % (w * nbuf) == 0 or (q // w) % nbuf == 0
    nwrite = q // w
    r = P * q
    nc = bass.Bass()
    idx = nc.dram_tensor("idx", [r], mybir.dt.int32, kind="ExternalInput")
    table = nc.dram_tensor("table", [vocab, D], mybir.dt.float32, kind="ExternalInput")
    out = nc.dram_tensor("out", [r, D], mybir.dt.float32, kind="ExternalOutput")

    idx_v = idx[:].rearrange("(p q) -> p q", p=P)          # [128, q]
    out_v = out[:].rearrange("(p q) d -> p q d", p=P)      # [128, q, 64]

    with contextlib.ExitStack() as ctx:
        idx_sb = ctx.enter_context(nc.sbuf_tensor([P, q], mybir.dt.int32))
        bufs = [
            ctx.enter_context(
                nc.sbuf_tensor(f"buf{i}", [P, w * D], mybir.dt.float32)
            )
            for i in range(nbuf)
        ]
        idx_sem = ctx.enter_context(nc.semaphore())
        # per-buffer sems: every wait targets the newest op issued on its sem
        gb_sems = [
            ctx.enter_context(nc.semaphore(name=f"gb_sem{i}")) for i in range(nbuf)
        ]
        wb_sems = [
            ctx.enter_context(nc.semaphore(name=f"wb_sem{i}")) for i in range(nbuf)
        ]
        block = ctx.enter_context(nc.Block())

        @block.sync
        def _(s):
            s.dma_start(idx_sb[:], idx_v).then_inc(idx_sem, 16)
            for wr in range(nwrite):
                b = wr % nbuf
                s.wait_ge(gb_sems[b], (wr // nbuf + 1) * w * 16)
                s.dma_start(out_v[:, wr * w:(wr + 1) * w, :], bufs[b][:]).then_inc(
                    wb_sems[b], 16
                )

        @block.gpsimd
        def _(gp):
            gp.wait_ge(idx_sem, 16)
            for c in range(q):
                wr = c // w
                b = wr % nbuf
                j = c % w
                if j == 0 and wr >= nbuf:
                    gp.wait_ge(wb_sems[b], (wr // nbuf) * 16)
                gp.indirect_dma_start(
                    out=bufs[b][:, j * D:(j + 1) * D],
                    out_offset=None,
                    in_=table[:],
                    in_offset=bass.IndirectOffsetOnAxis(
                        ap=idx_sb[:, c:c + 1], axis=0
                    ),
                ).then_inc(gb_sems[b], 16)

    return nc


def kernel(indices, table, dummy):
    global _compiled
    if _compiled is None:
        _compiled = build()
    nc = _compiled

    idx_flat = np.asarray(indices).astype(np.int32).reshape(-1)  # values < 1e6 fit
    table_np = np.ascontiguousarray(np.asarray(table, dtype=np.float32))

    in_maps = [
        {
            "idx": np.ascontiguousarray(idx_flat[c * R:(c + 1) * R]),
            "table": table_np,
        }
        for c in range(N_CORES)
    ]
    res = run_bass_kernel_spmd(nc, in_maps, core_ids=list(range(N_CORES)))
    out = np.concatenate([r["out"] for r in res.results], axis=0)
    return out.reshape(B, L, D)
